# revision 17
# baseline (speedup 1.0000x reference)
"""Trainium2 Bass kernel: sigmoid(rowdot(tanh(x1@W.T+b), tanh(x2@W.T+b))).

Sharding: pure data-parallel over batch across 8 NeuronCores (B=65536 ->
8192 rows/core, D_IN=1024, D_PROJ=128).

Key optimizations over the fp32 streaming version (~204 us):
  1. fp16 activations in HBM. The kernel is HBM-bound; x1/x2 are
     downcast to fp16 on the host, halving per-core traffic from 64 MiB
     to 32 MiB (~93 us floor at ~358 GB/s per core). fp16 keeps a
     10-bit mantissa -- same as the float32r (TF32) matmul datapath the
     fp32 version already used -- so end-to-end max rel err stays ~6e-3,
     well inside the 2e-2 gate (bf16's 8-bit mantissa would not).
  2. Host-side pre-transpose. x.T is uploaded with the contraction dim
     on partitions, removing ALL PE transpose-mode work (previously
     ~196K PE cycles vs 131K cycles of real matmul); PE now only does
     the real matmuls and stays under the DMA floor.
  3. Superblock-contiguous HBM layout. Each superblock of each tensor
     is stored [P, KC*rows] contiguous per partition, so loads are
     4-8 KiB descriptor runs per partition and stream at the HBM limit
     (~360 GB/s). A [p, k, B] layout produced 1-2 KiB chunks and
     measured only ~312 GB/s.
  4. HAM-aware granularity. 256-wide matmul j-blocks + <=1 MiB loads
     keep PE idle slices ~1-2 us (under the ~3.4 us HAM re-throttle
     window), so matmuls issue at the warm 2.4 GHz rate (measured
     109 ns/MM for N=256, LDWEIGHTS fully hidden) and PE tracks the
     DMA stream instead of oscillating cold.
  5. Ramp/drain shaping. SDMA round-robins all in-flight loads at
     packet granularity, so the FIRST load's completion is delayed by
     everything queued behind it (measured: first matmul gated ~10 us
     after issue with 1 MiB head blocks). The first/last two
     superblocks are 256 rows so the pipeline fills and drains fast.
     Consts go on the Scalar HWDGE queue; outputs are batched per
     superblock (one small store each).

Per-core dataflow, per 256-row j-block:
  PE:  po1[j,b] += wt_k.T @ x1t_k  (8 chunks, fp16, fp32 PSUM accum)
       po2[j,b] += wt_k.T @ x2t_k
  ACT: t = tanh(po + bias)  fused PSUM->SBUF, fp16 out
  DVE: prod = t1 * t2  (fp16, 2x DVE throughput)
  PE:  psim = ones.T @ prod  (partition reduction; deferred two j-blocks
       and emitted inside a later chain so PE never waits on tanh->mul)
  ACT: sigmoid -> column range of the superblock sig tile; one batched
       DMA store per superblock from sig row 0.
"""

import numpy as np

import concourse.bacc as bacc
import concourse.mybir as mybir
import concourse.tile as tile
from concourse.bass_utils import run_bass_kernel_spmd

N_CORES = 8
B_TOTAL = 65536
BSH = B_TOTAL // N_CORES  # 8192 rows per core
D_IN = 1024
D_PROJ = 128
P = 128
JB = 256                  # matmul moving-dim block
KC = D_IN // P            # 8 contraction chunks
# Superblock row counts: small at the ends for fast ramp-in/drain.
SBLOCKS = [64, 64, 128, 256] + [512] * 14 + [256, 128, 64, 64]
assert sum(SBLOCKS) == BSH

F32 = mybir.dt.float32
F16 = mybir.dt.float16


def _build_module():
    nc = bacc.Bacc("TRN2", target_bir_lowering=False, debug=False)

    x1t = nc.dram_tensor("x1t", [P, KC * BSH], F16, kind="ExternalInput").ap()
    x2t = nc.dram_tensor("x2t", [P, KC * BSH], F16, kind="ExternalInput").ap()
    wt = nc.dram_tensor("wt", [P, KC, D_PROJ], F16, kind="ExternalInput").ap()
    bias = nc.dram_tensor("bias", [P, 1], F32, kind="ExternalInput").ap()
    ones = nc.dram_tensor("ones", [P, P], F16, kind="ExternalInput").ap()
    out = nc.dram_tensor("out", [BSH], F32, kind="ExternalOutput").ap()

    with tile.TileContext(nc) as tc:
        with (
            tc.tile_pool(name="consts", bufs=1) as cpool,
            tc.tile_pool(name="xload", bufs=3) as xpool,
            tc.tile_pool(name="acts", bufs=2) as apool,
            tc.tile_pool(name="prods", bufs=3) as ppool,
            tc.tile_pool(name="sigs", bufs=2) as gpool,
            tc.tile_pool(name="po", bufs=4, space="PSUM") as opool,
            tc.tile_pool(name="psim", bufs=2, space="PSUM") as spool,
        ):
            # consts on the Scalar HWDGE queue: keeps the Sync queue free
            # so the x-stream starts issuing immediately.
            wt_sb = cpool.tile([P, KC, D_PROJ], F16, tag="wt")
            nc.scalar.dma_start(out=wt_sb, in_=wt)
            bias_sb = cpool.tile([P, 1], F32, tag="bias")
            nc.scalar.dma_start(out=bias_sb, in_=bias)
            ones_sb = cpool.tile([P, P], F16, tag="ones")
            nc.scalar.dma_start(out=ones_sb, in_=ones)

            # Deferred j-block tails (rowdot reduce + sigmoid). Entries
            # flush two chains later so PE never waits on tanh->mul.
            pending = []
            # superblock id -> [sig_tile, jj_done, nj, row0, nrows]
            sb_state = {}

            def flush_one():
                prod_p, sb_p, col_p, jbs_p = pending.pop(0)
                st = sb_state[sb_p]
                psim = spool.tile([P, 512], F32, name="psim", tag="psim")
                nc.tensor.matmul(
                    psim[:, :jbs_p],
                    ones_sb,
                    prod_p,
                    start=True,
                    stop=True,
                    skip_group_check=True,
                )
                nc.scalar.activation(
                    st[0][:, col_p:col_p + jbs_p],
                    psim[:, :jbs_p],
                    mybir.ActivationFunctionType.Sigmoid,
                )
                st[1] += 1
                if st[1] == st[2]:  # all j-blocks of superblock done
                    row0_s, nrows_s = st[3], st[4]
                    # store on the Scalar queue: the wait on the sigmoid is
                    # free there (sigmoids run on Scalar); on Sync it would
                    # stall all subsequent x-load issues (measured +40 us).
                    nc.scalar.dma_start(
                        out=out[row0_s:row0_s + nrows_s].rearrange(
                            "(a n) -> a n", a=1
                        ),
                        in_=st[0][0:1, 0:nrows_s],
                    )
                    del sb_state[sb_p]

            row0 = 0
            for sb, rows in enumerate(SBLOCKS):
                seg = KC * rows
                off = KC * row0
                x1n = xpool.tile([P, seg], F16, tag="x1")
                nc.sync.dma_start(out=x1n, in_=x1t[:, off:off + seg])
                x2n = xpool.tile([P, seg], F16, tag="x2")
                nc.sync.dma_start(out=x2n, in_=x2t[:, off:off + seg])
                jbs = min(rows, JB)
                nj = rows // jbs
                sig_sb = gpool.tile([P, 512], F32, name="sig", tag="sig")
                sb_state[sb] = [sig_sb, 0, nj, row0, rows]

                for j in range(nj):
                    po1 = opool.tile([P, 512], F32, name="po1", tag="po")
                    po2 = opool.tile([P, 512], F32, name="po2", tag="po")
                    for k in range(KC):
                        o = k * rows + j * jbs
                        nc.tensor.matmul(
                            po1[:, :jbs],
                            wt_sb[:, k, :],
                            x1n[:, o:o + jbs],
                            start=(k == 0),
                            stop=(k == KC - 1),
                            skip_group_check=True,
                        )
                        nc.tensor.matmul(
                            po2[:, :jbs],
                            wt_sb[:, k, :],
                            x2n[:, o:o + jbs],
                            start=(k == 0),
                            stop=(k == KC - 1),
                            skip_group_check=True,
                        )
                        if k == 2:
                            # normally keep one tail deferred; in the last
                            # two superblocks flush everything so only one
                            # j-block tail remains after the final chain.
                            thr = 0 if sb >= len(SBLOCKS) - 2 else 1
                            while len(pending) > thr:
                                flush_one()  # tail of j-block idx-2 rides here
                    t1 = apool.tile([P, jbs], F16, tag="t1")
                    nc.scalar.activation(
                        t1, po1[:, :jbs], mybir.ActivationFunctionType.Tanh,
                        bias=bias_sb,
                    )
                    t2 = apool.tile([P, jbs], F16, tag="t2")
                    nc.scalar.activation(
                        t2, po2[:, :jbs], mybir.ActivationFunctionType.Tanh,
                        bias=bias_sb,
                    )
                    prod = ppool.tile([P, jbs], F16, tag="prod")
                    nc.vector.tensor_mul(prod, t1, t2)
                    pending.append((prod, sb, j * jbs, jbs))
                row0 += rows
            while pending:
                flush_one()

    nc.compile()
    return nc


_NC_CACHE = None


def _get_module():
    global _NC_CACHE
    if _NC_CACHE is None:
        _NC_CACHE = _build_module()
    return _NC_CACHE


def _prep_inputs(x1, x2, W, b):
    """Host-side prep: fp16 downcast + transpose into the device layout.

    Device layout for x is [P, KC*BSH] fp16, a concatenation of
    per-superblock segments [P, KC, rows] with element (p, k, r) =
    x[row0 + r, k*128 + p]: contraction on partitions, each superblock
    contiguous per partition.
    """
    def xform(x):
        xs = np.asarray(x, dtype=np.float16).reshape(N_CORES, BSH, KC, P)
        parts = []
        off = 0
        for rows in SBLOCKS:
            blk = xs[:, off:off + rows]  # [C, rows, KC, P]
            parts.append(
                blk.transpose(0, 3, 2, 1).reshape(N_CORES, P, KC * rows)
            )
            off += rows
        return np.ascontiguousarray(np.concatenate(parts, axis=2))

    x1t = xform(x1)
    x2t = xform(x2)
    wt = np.ascontiguousarray(
        np.asarray(W, dtype=np.float16).T.reshape(KC, P, D_PROJ).transpose(1, 0, 2)
    )
    bias = np.ascontiguousarray(np.asarray(b, dtype=np.float32).reshape(P, 1))
    ones = np.ones((P, P), dtype=np.float16)
    return [
        {
            "x1t": x1t[i],
            "x2t": x2t[i],
            "wt": wt,
            "bias": bias,
            "ones": ones,
        }
        for i in range(N_CORES)
    ]


def kernel(x1, x2, W, b):
    nc = _get_module()
    in_maps = _prep_inputs(x1, x2, W, b)
    res = run_bass_kernel_spmd(nc, in_maps, core_ids=list(range(N_CORES)))
    return np.concatenate([res.results[i]["out"] for i in range(N_CORES)])


# revision 18
# speedup vs baseline: 1.0405x; 1.0405x over previous
"""Trainium2 Bass kernel: sigmoid(rowdot(tanh(x1@W.T+b), tanh(x2@W.T+b))).

Sharding: pure data-parallel over batch across 8 NeuronCores (B=65536 ->
8192 rows/core, D_IN=1024, D_PROJ=128).

Key optimizations over the fp32 streaming version (~204 us):
  1. fp16 activations in HBM. The kernel is HBM-bound; x1/x2 are
     downcast to fp16 on the host, halving per-core traffic from 64 MiB
     to 32 MiB (~93 us floor at ~358 GB/s per core). fp16 keeps a
     10-bit mantissa -- same as the float32r (TF32) matmul datapath the
     fp32 version already used -- so end-to-end max rel err stays ~6e-3,
     well inside the 2e-2 gate (bf16's 8-bit mantissa would not).
  2. Host-side pre-transpose. x.T is uploaded with the contraction dim
     on partitions, removing ALL PE transpose-mode work (previously
     ~196K PE cycles vs 131K cycles of real matmul); PE now only does
     the real matmuls and stays under the DMA floor.
  3. Superblock-contiguous HBM layout. Each superblock of each tensor
     is stored [P, KC*rows] contiguous per partition, so loads are
     4-8 KiB descriptor runs per partition and stream at the HBM limit
     (~360 GB/s). A [p, k, B] layout produced 1-2 KiB chunks and
     measured only ~312 GB/s.
  4. HAM-aware granularity. 256-wide matmul j-blocks + <=1 MiB loads
     keep PE idle slices ~1-2 us (under the ~3.4 us HAM re-throttle
     window), so matmuls issue at the warm 2.4 GHz rate (measured
     109 ns/MM for N=256, LDWEIGHTS fully hidden) and PE tracks the
     DMA stream instead of oscillating cold.
  5. Ramp/drain shaping. SDMA round-robins all in-flight loads at
     packet granularity, so the FIRST load's completion is delayed by
     everything queued behind it (measured: first matmul gated ~10 us
     after issue with 1 MiB head blocks). The first/last two
     superblocks are 256 rows so the pipeline fills and drains fast.
     Consts go on the Scalar HWDGE queue; outputs are batched per
     superblock (one small store each).

Per-core dataflow, per 256-row j-block:
  PE:  po1[j,b] += wt_k.T @ x1t_k  (8 chunks, fp16, fp32 PSUM accum)
       po2[j,b] += wt_k.T @ x2t_k
  ACT: t = tanh(po + bias)  fused PSUM->SBUF, fp16 out
  DVE: prod = t1 * t2  (fp16, 2x DVE throughput)
  PE:  psim = ones.T @ prod  (partition reduction; deferred two j-blocks
       and emitted inside a later chain so PE never waits on tanh->mul)
  ACT: sigmoid -> column range of the superblock sig tile; one batched
       DMA store per superblock from sig row 0.
"""

import numpy as np

import concourse.bacc as bacc
import concourse.mybir as mybir
import concourse.tile as tile
from concourse.bass_utils import run_bass_kernel_spmd

N_CORES = 8
B_TOTAL = 65536
BSH = B_TOTAL // N_CORES  # 8192 rows per core
D_IN = 1024
D_PROJ = 128
P = 128
JB = 256                  # matmul moving-dim block
KC = D_IN // P            # 8 contraction chunks
# Superblock row counts: small at the ends for fast ramp-in/drain.
SBLOCKS = [128, 128, 256] + [512] * 14 + [256, 128, 128]
assert sum(SBLOCKS) == BSH

F32 = mybir.dt.float32
F16 = mybir.dt.float16


def _build_module():
    nc = bacc.Bacc("TRN2", target_bir_lowering=False, debug=False)

    x1t = nc.dram_tensor("x1t", [P, KC * BSH], F16, kind="ExternalInput").ap()
    x2t = nc.dram_tensor("x2t", [P, KC * BSH], F16, kind="ExternalInput").ap()
    wt = nc.dram_tensor("wt", [P, KC, D_PROJ], F16, kind="ExternalInput").ap()
    bias = nc.dram_tensor("bias", [P, 1], F32, kind="ExternalInput").ap()
    ones = nc.dram_tensor("ones", [P, P], F16, kind="ExternalInput").ap()
    out = nc.dram_tensor("out", [BSH], F32, kind="ExternalOutput").ap()

    with tile.TileContext(nc) as tc:
        with (
            tc.tile_pool(name="consts", bufs=1) as cpool,
            tc.tile_pool(name="xload", bufs=3) as xpool,
            tc.tile_pool(name="acts", bufs=2) as apool,
            tc.tile_pool(name="prods", bufs=3) as ppool,
            tc.tile_pool(name="sigs", bufs=2) as gpool,
            tc.tile_pool(name="po", bufs=4, space="PSUM") as opool,
            tc.tile_pool(name="psim", bufs=2, space="PSUM") as spool,
        ):
            # consts on the Scalar HWDGE queue: keeps the Sync queue free
            # so the x-stream starts issuing immediately.
            wt_sb = cpool.tile([P, KC, D_PROJ], F16, tag="wt")
            nc.scalar.dma_start(out=wt_sb, in_=wt)
            bias_sb = cpool.tile([P, 1], F32, tag="bias")
            nc.scalar.dma_start(out=bias_sb, in_=bias)
            ones_sb = cpool.tile([P, P], F16, tag="ones")
            nc.scalar.dma_start(out=ones_sb, in_=ones)

            # Deferred j-block tails (rowdot reduce + sigmoid). Entries
            # flush two chains later so PE never waits on tanh->mul.
            pending = []
            # superblock id -> [sig_tile, jj_done, nj, row0, nrows]
            sb_state = {}

            def flush_one():
                prod_p, sb_p, col_p, jbs_p = pending.pop(0)
                st = sb_state[sb_p]
                psim = spool.tile([P, 512], F32, name="psim", tag="psim")
                nc.tensor.matmul(
                    psim[:, :jbs_p],
                    ones_sb,
                    prod_p,
                    start=True,
                    stop=True,
                    skip_group_check=True,
                )
                nc.scalar.activation(
                    st[0][:, col_p:col_p + jbs_p],
                    psim[:, :jbs_p],
                    mybir.ActivationFunctionType.Sigmoid,
                )
                st[1] += 1
                if st[1] == st[2]:  # all j-blocks of superblock done
                    row0_s, nrows_s = st[3], st[4]
                    # store on the Scalar queue: the wait on the sigmoid is
                    # free there (sigmoids run on Scalar); on Sync it would
                    # stall all subsequent x-load issues (measured +40 us).
                    nc.scalar.dma_start(
                        out=out[row0_s:row0_s + nrows_s].rearrange(
                            "(a n) -> a n", a=1
                        ),
                        in_=st[0][0:1, 0:nrows_s],
                    )
                    del sb_state[sb_p]

            row0 = 0
            for sb, rows in enumerate(SBLOCKS):
                seg = KC * rows
                off = KC * row0
                x1n = xpool.tile([P, seg], F16, tag="x1")
                nc.sync.dma_start(out=x1n, in_=x1t[:, off:off + seg])
                x2n = xpool.tile([P, seg], F16, tag="x2")
                nc.sync.dma_start(out=x2n, in_=x2t[:, off:off + seg])
                jbs = min(rows, JB)
                nj = rows // jbs
                sig_sb = gpool.tile([P, 512], F32, name="sig", tag="sig")
                sb_state[sb] = [sig_sb, 0, nj, row0, rows]

                for j in range(nj):
                    po1 = opool.tile([P, 512], F32, name="po1", tag="po")
                    po2 = opool.tile([P, 512], F32, name="po2", tag="po")
                    for k in range(KC):
                        o = k * rows + j * jbs
                        nc.tensor.matmul(
                            po1[:, :jbs],
                            wt_sb[:, k, :],
                            x1n[:, o:o + jbs],
                            start=(k == 0),
                            stop=(k == KC - 1),
                            skip_group_check=True,
                        )
                        nc.tensor.matmul(
                            po2[:, :jbs],
                            wt_sb[:, k, :],
                            x2n[:, o:o + jbs],
                            start=(k == 0),
                            stop=(k == KC - 1),
                            skip_group_check=True,
                        )
                        if k == 2:
                            # normally keep one tail deferred; in the last
                            # two superblocks flush everything so only one
                            # j-block tail remains after the final chain.
                            thr = 0 if sb >= len(SBLOCKS) - 2 else 1
                            while len(pending) > thr:
                                flush_one()  # tail of j-block idx-2 rides here
                    t1 = apool.tile([P, jbs], F16, tag="t1")
                    nc.scalar.activation(
                        t1, po1[:, :jbs], mybir.ActivationFunctionType.Tanh,
                        bias=bias_sb,
                    )
                    t2 = apool.tile([P, jbs], F16, tag="t2")
                    nc.scalar.activation(
                        t2, po2[:, :jbs], mybir.ActivationFunctionType.Tanh,
                        bias=bias_sb,
                    )
                    prod = ppool.tile([P, jbs], F16, tag="prod")
                    nc.vector.tensor_mul(prod, t1, t2)
                    pending.append((prod, sb, j * jbs, jbs))
                row0 += rows
            while pending:
                flush_one()

    nc.compile()
    return nc


_NC_CACHE = None


def _get_module():
    global _NC_CACHE
    if _NC_CACHE is None:
        _NC_CACHE = _build_module()
    return _NC_CACHE


def _prep_inputs(x1, x2, W, b):
    """Host-side prep: fp16 downcast + transpose into the device layout.

    Device layout for x is [P, KC*BSH] fp16, a concatenation of
    per-superblock segments [P, KC, rows] with element (p, k, r) =
    x[row0 + r, k*128 + p]: contraction on partitions, each superblock
    contiguous per partition.
    """
    def xform(x):
        xs = np.asarray(x, dtype=np.float16).reshape(N_CORES, BSH, KC, P)
        parts = []
        off = 0
        for rows in SBLOCKS:
            blk = xs[:, off:off + rows]  # [C, rows, KC, P]
            parts.append(
                blk.transpose(0, 3, 2, 1).reshape(N_CORES, P, KC * rows)
            )
            off += rows
        return np.ascontiguousarray(np.concatenate(parts, axis=2))

    x1t = xform(x1)
    x2t = xform(x2)
    wt = np.ascontiguousarray(
        np.asarray(W, dtype=np.float16).T.reshape(KC, P, D_PROJ).transpose(1, 0, 2)
    )
    bias = np.ascontiguousarray(np.asarray(b, dtype=np.float32).reshape(P, 1))
    ones = np.ones((P, P), dtype=np.float16)
    return [
        {
            "x1t": x1t[i],
            "x2t": x2t[i],
            "wt": wt,
            "bias": bias,
            "ones": ones,
        }
        for i in range(N_CORES)
    ]


def kernel(x1, x2, W, b):
    nc = _get_module()
    in_maps = _prep_inputs(x1, x2, W, b)
    res = run_bass_kernel_spmd(nc, in_maps, core_ids=list(range(N_CORES)))
    return np.concatenate([res.results[i]["out"] for i in range(N_CORES)])
